# revision 19
# baseline (speedup 1.0000x reference)
"""TRN2 Bass kernel for nn_Attention_59270548685139.

Custom two-stage-normalized attention, B=8, N=1024, D=1024, H=8, DH=64.
Sharding: data-parallel over batch -- one batch element per NeuronCore (8 cores).

Math per batch element (matching the reference):
  q = x @ Wq, k = x @ Wk, v = x @ Wv          (split into 8 heads of 64)
  sim[i,j]  = (q_i . k_j) * DH**-0.5
  attn      = softmax over the QUERY dim i    -> E[i,j]/C[j], C[j] = sum_i E[i,j]
  attn      = attn / (sum_j attn + eps)       -> per-i scale 1/(R[i]+eps)
  out       = attn @ v ; y = out @ Wo + bo

Schedule (v3):
- Head-PAIR loop with row-tiled concurrent score matmuls (lhsT base
  partitions 0/64 -> ~2x); exp -> E^T in bf16 with C from the ACT
  accumulator; GPSIMD folds 1/C into V (+ appended 1/C column so attn@v
  also yields R in row 64).
- Score PSUM ring is 3 deep (6 banks) and runs 1.5 steps ahead of the
  exps, so ACT never waits on the PE; attn@V runs as two 8-link chains
  per head at the tail of its exp stream (2 banks), hidden under the
  other head's / next pair's exps.
- PSUM drains ride GPSIMD; the DVE only does the reciprocal (split in
  halves to bound head-of-line latency), the normalization multiply and
  the final bias-add, so no engine queue backs up.
- V projection fills pair-0's PE slack, q/k projections for pair p+1
  fill pair p's; paced warm-up matmuls keep the PE HAM monitor busy
  through the initial x DMA.
- Output projection: Wo chunks mb0-2 for the first blocks start before
  the last head's normalization lands; bo is added during the
  PSUM->SBUF drain (DVE tensor-tensor add, no K=1 bias matmuls).
"""

import os

import numpy as np

import concourse.bass as bass
import concourse.tile as tile
from concourse import bacc, mybir
from concourse.bass_utils import run_bass_kernel_spmd
from concourse.masks import make_identity

FP32 = mybir.dt.float32
FP32R = mybir.dt.float32r
BF16 = mybir.dt.bfloat16

B, N, D = 8, 1024, 1024
H, DH = 8, 64
INNER = H * DH  # 512
SCALE = DH ** -0.5
EPS = 1e-7  # negligible vs R in [0.85, 1.15]; folded out
P = 128
NCORES = 8

_NC_CACHE = None


def _build_nc():
    nc = bacc.Bacc("TRN2", target_bir_lowering=False, debug=False)

    x_d = nc.dram_tensor("x", [N, D], FP32, kind="ExternalInput")
    wq_d = nc.dram_tensor("Wq", [D, INNER], FP32, kind="ExternalInput")
    wk_d = nc.dram_tensor("Wk", [D, INNER], FP32, kind="ExternalInput")
    wv_d = nc.dram_tensor("Wv", [D, INNER], FP32, kind="ExternalInput")
    wo_d = nc.dram_tensor("Wo", [INNER, D], FP32, kind="ExternalInput")
    bo_d = nc.dram_tensor("bo", [D], FP32, kind="ExternalInput")
    y_d = nc.dram_tensor("y", [N, D], FP32, kind="ExternalOutput")

    DC = D // P       # 8 contraction chunks over D
    IC = INNER // P   # 4 chunks over INNER
    NB = N // P       # 8 seq blocks of 128

    with tile.TileContext(nc) as tc:
        const_pool = tc.alloc_tile_pool(name="const", bufs=1)
        xt_pool = tc.alloc_tile_pool(name="xt", bufs=1)
        qt_pool = tc.alloc_tile_pool(name="qt", bufs=1)
        kt_pool = tc.alloc_tile_pool(name="kt", bufs=1)
        v_pool = tc.alloc_tile_pool(name="v", bufs=1)
        ot_pool = tc.alloc_tile_pool(name="ot", bufs=1)
        wv_pool = tc.alloc_tile_pool(name="wv", bufs=1)
        w4_pool = tc.alloc_tile_pool(name="w4", bufs=4)
        xn_pool = tc.alloc_tile_pool(name="xn", bufs=3)
        et_pool = tc.alloc_tile_pool(name="et", bufs=1)
        v2_pool = tc.alloc_tile_pool(name="v2", bufs=1)
        c_pool = tc.alloc_tile_pool(name="cp", bufs=1)
        us_pool = tc.alloc_tile_pool(name="us", bufs=2)
        sm_pool = tc.alloc_tile_pool(name="sm", bufs=2)
        y_pool = tc.alloc_tile_pool(name="yp", bufs=2)
        ps_pool = tc.alloc_tile_pool(name="ps", bufs=2, space="PSUM")

        # ---------------- constants ----------------
        ident = const_pool.tile([P, P], FP32, tag="ident")
        make_identity(nc, ident[:])
        # bo row + broadcast (added during the y drain).  Weight/bias DMAs
        # ride the scalar-engine queue so the x stream never waits on them.
        bo_row = const_pool.tile([1, D], FP32, tag="bo_row")
        nc.scalar.dma_start(out=bo_row[:], in_=bo_d.ap()[None, :])
        bo_bc = const_pool.tile([P, D], FP32, tag="bo_bc")
        nc.gpsimd.partition_broadcast(bo_bc[:], bo_row[:])

        # ---------------- persistent intermediates ----------------
        xt = [xt_pool.tile([P, N], FP32R, tag=f"xt{c}", name=f"xt{c}") for c in range(DC)]
        qt = [qt_pool.tile([P, N], FP32R, tag=f"qt{m}", name=f"qt{m}") for m in range(IC)]
        kt = [kt_pool.tile([P, N], FP32R, tag=f"kt{m}", name=f"kt{m}") for m in range(IC)]
        vts = [v_pool.tile([P, INNER], FP32, tag=f"v{j}", name=f"v{j}") for j in range(NB)]
        ot = [ot_pool.tile([P, N], FP32R, tag=f"ot{m}", name=f"ot{m}") for m in range(IC)]

        def load_qk_quarter(key, wd, mb, eng=None):
            w4 = w4_pool.tile([P, DC, P], FP32R, tag="w4", name=f"w4{key}{mb}")
            (eng or nc.scalar).dma_start(
                out=w4[:],
                in_=wd.ap()[:, mb * P:(mb + 1) * P]
                .rearrange("(c p) n -> p c n", p=P).bitcast(FP32R),
            )
            return w4

        # ---------------- preamble ----------------
        w4q = {}

        def emit_x_block(ib, n_warm):
            halves = []
            for hh in range(2):
                xh = xn_pool.tile([P, 512], FP32, tag="xn", name=f"xn{ib}_{hh}")
                nc.sync.dma_start(
                    out=xh[:],
                    in_=x_d.ap()[ib * P:(ib + 1) * P, hh * 512:(hh + 1) * 512],
                )
                halves.append(xh)
            # paced warm-up matmuls on landed data (results unused): keep the
            # PE HAM activity monitor busy through the DMA phase.
            p_w = ps_pool.tile([P, 512], FP32, tag="u", name=f"wu{ib}")
            for w in range(n_warm):
                nc.tensor.matmul(
                    p_w[:, (w % 4) * P:(w % 4 + 1) * P], ident[:],
                    halves[w % 2][:, 0:P], start=True, stop=True,
                )
            p_t = ps_pool.tile([P, N], FP32, tag="big", name=f"ptp{ib}", bufs=3)
            for c in range(DC):
                nc.tensor.transpose(
                    p_t[:, c * P:(c + 1) * P],
                    halves[c // 4][:, (c % 4) * P:(c % 4 + 1) * P],
                    ident[:],
                )
            for c in range(DC):
                if c % 2 == 0:
                    nc.scalar.copy(xt[c][:, ib * P:(ib + 1) * P], p_t[:, c * P:(c + 1) * P])
                else:
                    nc.vector.tensor_copy(xt[c][:, ib * P:(ib + 1) * P], p_t[:, c * P:(c + 1) * P])

        def emit_qk_proj_half(key, dst, mb, ih):
            w4 = w4q[(key, mb)]
            p_t = ps_pool.tile([P, N], FP32, tag="big", name=f"pp{key}{mb}_{ih}", bufs=3)
            for c in range(DC):
                nc.tensor.matmul(
                    p_t[:, 0:512],
                    w4[:, c, :],
                    xt[c][:, ih * 512:(ih + 1) * 512],
                    start=(c == 0), stop=(c == DC - 1),
                )
            nc.vector.tensor_copy(dst[mb][:, ih * 512:(ih + 1) * 512], p_t[:, 0:512])

        def emit_v_proj(jb):
            p_t = ps_pool.tile([P, N], FP32, tag="big", name=f"pv{jb}", bufs=3)
            for c in range(DC):
                nc.tensor.matmul(
                    p_t[:, 0:512],
                    xt[c][:, jb * P:(jb + 1) * P],
                    wv_t[:, c, :],
                    start=(c == 0), stop=(c == DC - 1),
                )
            nc.vector.tensor_copy(vts[jb][:], p_t[:, 0:512])

        w4q[("q", 0)] = load_qk_quarter("q", wq_d, 0)
        w4q[("k", 0)] = load_qk_quarter("k", wk_d, 0)
        wv_t = wv_pool.tile([P, DC, INNER], FP32R, tag="wv")
        nc.scalar.dma_start(
            out=wv_t[:],
            in_=wv_d.ap().rearrange("(c p) n -> p c n", p=P).bitcast(FP32R),
        )
        for ib in range(4):
            emit_x_block(ib, n_warm=4)
        emit_qk_proj_half("q", qt, 0, 0)
        emit_qk_proj_half("k", kt, 0, 0)
        emit_v_proj(0)
        for ib in range(4, 8):
            emit_x_block(ib, n_warm=2)
        emit_v_proj(1)
        emit_qk_proj_half("q", qt, 0, 1)
        emit_qk_proj_half("k", kt, 0, 1)
        w4q[("q", 1)] = load_qk_quarter("q", wq_d, 1)
        w4q[("k", 1)] = load_qk_quarter("k", wk_d, 1)

        # ---------------- head-pair loop ----------------
        finish_args = {}

        def emit_finish(h):
            mb, off = h // 2, (h % 2) * DH
            us = finish_args.pop(h)
            rrec = sm_pool.tile([1, N], FP32, tag="rrec", name=f"rrec{h}")
            bc = sm_pool.tile([DH, N], FP32, tag="bc", name=f"bc{h}")
            for ih in range(2):
                sl = slice(ih * 512, (ih + 1) * 512)
                nc.vector.reciprocal(rrec[:, sl], us[DH:DH + 1, sl])
                nc.gpsimd.partition_broadcast(bc[:, sl], rrec[:, sl])
                nc.vector.tensor_mul(ot[mb][off:off + DH, sl], us[0:DH, sl], bc[:, sl])

        pair_state = {}

        def emit_scores(p, jb):
            mb = p
            psA = ps_pool.tile([P, N], FP32, tag="big", name=f"sA{p}_{jb}", bufs=3)
            psB = ps_pool.tile([P, N], FP32, tag="big", name=f"sB{p}_{jb}", bufs=3)
            for ih in range(2):
                nc.tensor.matmul(
                    psA[:, ih * 512:(ih + 1) * 512],
                    kt[mb][0:DH, jb * P:(jb + 1) * P],
                    qt[mb][0:DH, ih * 512:(ih + 1) * 512],
                    start=True, stop=True,
                )
                nc.tensor.matmul(
                    psB[:, ih * 512:(ih + 1) * 512],
                    kt[mb][DH:P, jb * P:(jb + 1) * P],
                    qt[mb][DH:P, ih * 512:(ih + 1) * 512],
                    start=True, stop=True,
                )
            pair_state[(p, jb)] = (psA, psB)

        def emit_chains(p, par):
            """Both i-half attn@V chains for head (2p + par) + GPSIMD drain."""
            st = pair_state[("tiles", p)]
            v2t = st["v2A"] if par == 0 else st["v2B"]
            ett = st["etA"] if par == 0 else st["etB"]
            h = 2 * p + par
            us = us_pool.tile([DH + 1, N], FP32, tag="us", name=f"us{h}")
            for ih in range(2):
                pu = ps_pool.tile([P, 512], FP32, tag="u", name=f"pu{h}_{ih}")
                for jb in range(NB):
                    nc.tensor.matmul(
                        pu[0:DH + 1, :],
                        v2t[:, jb, 0:DH + 1],
                        ett[jb][:, ih * 512:(ih + 1) * 512],
                        start=(jb == 0), stop=(jb == NB - 1),
                    )
                nc.vector.tensor_copy(us[:, ih * 512:(ih + 1) * 512], pu[0:DH + 1, :])
            finish_args[h] = us

        for p in range(4):
            hA, hB = 2 * p, 2 * p + 1
            cA = c_pool.tile([P, NB], FP32, tag="cA", name=f"cA{p}")
            cB = c_pool.tile([P, NB], FP32, tag="cB", name=f"cB{p}")
            v2A = v2_pool.tile([P, NB, DH + 2], BF16, tag="v2A", name=f"v2A{p}")
            v2B = v2_pool.tile([P, NB, DH + 2], BF16, tag="v2B", name=f"v2B{p}")
            etA, etB = {}, {}
            pair_state[("tiles", p)] = {"v2A": v2A, "v2B": v2B, "etA": etA, "etB": etB}

            emit_scores(p, 0)
            emit_scores(p, 1)
            if p > 0:
                emit_chains(p - 1, 1)  # previous pair's B head, under our exps

            for jb in range(NB):
                psA, psB = pair_state.pop((p, jb))
                etA[jb] = et_pool.tile([P, N], BF16, tag=f"etA{jb}", name=f"etA{p}_{jb}")
                nc.scalar.activation(
                    etA[jb][:], psA[:], mybir.ActivationFunctionType.Exp,
                    scale=SCALE, accum_out=cA[:, jb:jb + 1],
                )
                etB[jb] = et_pool.tile([P, N], BF16, tag=f"etB{jb}", name=f"etB{p}_{jb}")
                nc.scalar.activation(
                    etB[jb][:], psB[:], mybir.ActivationFunctionType.Exp,
                    scale=SCALE, accum_out=cB[:, jb:jb + 1],
                )
                nc.gpsimd.normalize_recip(
                    v2A[:, jb, 0:DH], vts[jb][:, hA * DH:(hA + 1) * DH], cA[:, jb:jb + 1]
                )
                nc.gpsimd.tensor_copy(v2A[:, jb, DH:DH + 1], cA[:, jb:jb + 1])
                nc.gpsimd.normalize_recip(
                    v2B[:, jb, 0:DH], vts[jb][:, hB * DH:(hB + 1) * DH], cB[:, jb:jb + 1]
                )
                nc.gpsimd.tensor_copy(v2B[:, jb, DH:DH + 1], cB[:, jb:jb + 1])

                if jb < 6:
                    emit_scores(p, jb + 2)

                # PE filler
                if p == 0:
                    if jb < 6:
                        emit_v_proj(jb + 2)
                    if jb == 3:
                        emit_qk_proj_half("q", qt, 1, 0)
                    if jb == 4:
                        emit_qk_proj_half("k", kt, 1, 0)
                    if jb == 5:
                        emit_qk_proj_half("q", qt, 1, 1)
                elif p < 3:
                    if jb == 0:
                        emit_qk_proj_half("k", kt, p, 1)  # prev pair's spill
                    if jb == 1:
                        emit_qk_proj_half("q", qt, p + 1, 0)
                    if jb == 3:
                        emit_qk_proj_half("k", kt, p + 1, 0)
                    if jb == 5:
                        emit_qk_proj_half("q", qt, p + 1, 1)
                else:
                    if jb == 0:
                        emit_qk_proj_half("k", kt, 3, 1)

                if p > 0 and jb == 1:
                    emit_finish(2 * p - 2)
                if p > 0 and jb == 5:
                    emit_finish(2 * p - 1)

            emit_chains(p, 0)  # A head's chains, under B's exp stream

            if p == 0:
                w4q[("q", 2)] = load_qk_quarter("q", wq_d, 2, eng=nc.sync)
                w4q[("k", 2)] = load_qk_quarter("k", wk_d, 2, eng=nc.sync)
            if p == 1:
                w4q[("q", 3)] = load_qk_quarter("q", wq_d, 3, eng=nc.sync)
                w4q[("k", 3)] = load_qk_quarter("k", wk_d, 3, eng=nc.sync)

        # Wo quarters into freed w4 slots (sync queue, idle post-preamble)
        wo4 = []
        for mbi in range(IC):
            w4 = w4_pool.tile([P, D], FP32R, tag="w4", name=f"w4o{mbi}")
            nc.sync.dma_start(
                out=w4[:], in_=wo_d.ap()[mbi * P:(mbi + 1) * P, :].bitcast(FP32R)
            )
            wo4.append(w4)

        # ---------------- tail ----------------
        emit_finish(H - 2)        # head 6; DVE recip runs under the B chains
        emit_chains(3, 1)         # head 7 chains
        # output projection: first 3 blocks start on mb0-2 before head 7's
        # normalization lands; mb3 links + the rest follow.
        pys = {}
        for ib in range(3):
            p_y = ps_pool.tile([P, N], FP32, tag="big", name=f"py{ib}", bufs=3)
            for db in range(2):
                for mbi in range(3):
                    nc.tensor.matmul(
                        p_y[:, db * 512:(db + 1) * 512],
                        ot[mbi][:, ib * P:(ib + 1) * P],
                        wo4[mbi][:, db * 512:(db + 1) * 512],
                        start=(mbi == 0), stop=False,
                    )
            pys[ib] = p_y
        emit_finish(H - 1)        # head 7

        def drain_y(ib, p_y):
            for db in range(2):
                y_t = y_pool.tile([P, 512], FP32, tag="y", name=f"y{ib}_{db}")
                nc.vector.tensor_add(
                    y_t[:], p_y[:, db * 512:(db + 1) * 512],
                    bo_bc[:, db * 512:(db + 1) * 512],
                )
                nc.sync.dma_start(
                    out=y_d.ap()[ib * P:(ib + 1) * P, db * 512:(db + 1) * 512],
                    in_=y_t[:],
                )

        for ib in range(3):
            p_y = pys.pop(ib)
            for db in range(2):
                nc.tensor.matmul(
                    p_y[:, db * 512:(db + 1) * 512],
                    ot[3][:, ib * P:(ib + 1) * P],
                    wo4[3][:, db * 512:(db + 1) * 512],
                    start=False, stop=True,
                )
            drain_y(ib, p_y)
        for ib in range(3, NB):
            p_y = ps_pool.tile([P, N], FP32, tag="big", name=f"py{ib}", bufs=3)
            for db in range(2):
                for mbi in range(IC):
                    nc.tensor.matmul(
                        p_y[:, db * 512:(db + 1) * 512],
                        ot[mbi][:, ib * P:(ib + 1) * P],
                        wo4[mbi][:, db * 512:(db + 1) * 512],
                        start=(mbi == 0), stop=(mbi == IC - 1),
                    )
            drain_y(ib, p_y)

        for pool in (ps_pool, y_pool, sm_pool, us_pool, c_pool, v2_pool, et_pool,
                     xn_pool, w4_pool, wv_pool, ot_pool, v_pool, kt_pool, qt_pool,
                     xt_pool, const_pool):
            pool.release()

    nc.finalize()
    return nc


def _get_nc():
    global _NC_CACHE
    if _NC_CACHE is None:
        _NC_CACHE = _build_nc()
    return _NC_CACHE


def kernel(x, Wq, Wk, Wv, Wo, bo, _trace=False, **trace_kwargs):
    x = np.ascontiguousarray(np.asarray(x, dtype=np.float32))
    Wq = np.ascontiguousarray(np.asarray(Wq, dtype=np.float32))
    Wk = np.ascontiguousarray(np.asarray(Wk, dtype=np.float32))
    Wv = np.ascontiguousarray(np.asarray(Wv, dtype=np.float32))
    Wo = np.ascontiguousarray(np.asarray(Wo, dtype=np.float32))
    bo = np.ascontiguousarray(np.asarray(bo, dtype=np.float32))

    nc = _get_nc()
    in_maps = [
        {"x": x[c], "Wq": Wq, "Wk": Wk, "Wv": Wv, "Wo": Wo, "bo": bo}
        for c in range(NCORES)
    ]
    res = run_bass_kernel_spmd(
        nc, in_maps, core_ids=list(range(NCORES)), trace=_trace, **trace_kwargs
    )
    out = np.stack([res.results[c]["y"] for c in range(NCORES)], axis=0)
    if _trace:
        return out.astype(np.float32), res
    return out.astype(np.float32)


if __name__ == "__main__":
    rng = np.random.default_rng(0)
    xs = rng.standard_normal((B, N, D), dtype=np.float32)
    wq = rng.standard_normal((D, INNER), dtype=np.float32) * D ** -0.5
    wk = rng.standard_normal((D, INNER), dtype=np.float32) * D ** -0.5
    wv = rng.standard_normal((D, INNER), dtype=np.float32) * D ** -0.5
    wo = rng.standard_normal((INNER, D), dtype=np.float32) * INNER ** -0.5
    bz = np.zeros((D,), dtype=np.float32)
    y = kernel(xs, wq, wk, wv, wo, bz)
    print("ran ok", y.shape, float(np.abs(y).mean()))


# revision 22
# speedup vs baseline: 1.3105x; 1.3105x over previous
"""TRN2 Bass kernel for nn_Attention_59270548685139.

Custom two-stage-normalized attention, B=8, N=1024, D=1024, H=8, DH=64.
Sharding: data-parallel over batch -- one batch element per NeuronCore (8 cores).

Math per batch element (matching the reference):
  q = x @ Wq, k = x @ Wk, v = x @ Wv          (split into 8 heads of 64)
  sim[i,j]  = (q_i . k_j) * DH**-0.5
  attn      = softmax over the QUERY dim i    -> E[i,j]/C[j], C[j] = sum_i E[i,j]
  attn      = attn / (sum_j attn + eps)       -> per-i scale 1/(R[i]+eps)
  out       = attn @ v ; y = out @ Wo + bo

Schedule (v3):
- Head-PAIR loop with row-tiled concurrent score matmuls (lhsT base
  partitions 0/64 -> ~2x); exp -> E^T in bf16 with C from the ACT
  accumulator; GPSIMD folds 1/C into V (+ appended 1/C column so attn@v
  also yields R in row 64).
- Score PSUM ring is 3 deep (6 banks) and runs 1.5 steps ahead of the
  exps, so ACT never waits on the PE; attn@V runs as two 8-link chains
  per head at the tail of its exp stream (2 banks), hidden under the
  other head's / next pair's exps.
- PSUM drains ride GPSIMD; the DVE only does the reciprocal (split in
  halves to bound head-of-line latency), the normalization multiply and
  the final bias-add, so no engine queue backs up.
- V projection fills pair-0's PE slack, q/k projections for pair p+1
  fill pair p's; paced warm-up matmuls keep the PE HAM monitor busy
  through the initial x DMA.
- Output projection: Wo chunks mb0-2 for the first blocks start before
  the last head's normalization lands; bo is added during the
  PSUM->SBUF drain (DVE tensor-tensor add, no K=1 bias matmuls).
"""

import os

import numpy as np

import concourse.bass as bass
import concourse.tile as tile
from concourse import bacc, mybir
from concourse.bass_utils import run_bass_kernel_spmd
from concourse.masks import make_identity

FP32 = mybir.dt.float32
FP32R = mybir.dt.float32r
BF16 = mybir.dt.bfloat16

B, N, D = 8, 1024, 1024
H, DH = 8, 64
INNER = H * DH  # 512
SCALE = DH ** -0.5
EPS = 1e-7  # negligible vs R in [0.85, 1.15]; folded out
P = 128
NCORES = 8

_NC_CACHE = None


def _build_nc():
    nc = bacc.Bacc("TRN2", target_bir_lowering=False, debug=False)

    x_d = nc.dram_tensor("x", [N, D], FP32, kind="ExternalInput")
    wq_d = nc.dram_tensor("Wq", [D, INNER], FP32, kind="ExternalInput")
    wk_d = nc.dram_tensor("Wk", [D, INNER], FP32, kind="ExternalInput")
    wv_d = nc.dram_tensor("Wv", [D, INNER], FP32, kind="ExternalInput")
    wo_d = nc.dram_tensor("Wo", [INNER, D], FP32, kind="ExternalInput")
    bo_d = nc.dram_tensor("bo", [D], FP32, kind="ExternalInput")
    y_d = nc.dram_tensor("y", [N, D], FP32, kind="ExternalOutput")

    DC = D // P       # 8 contraction chunks over D
    IC = INNER // P   # 4 chunks over INNER
    NB = N // P       # 8 seq blocks of 128

    with tile.TileContext(nc) as tc:
        const_pool = tc.alloc_tile_pool(name="const", bufs=1)
        xt_pool = tc.alloc_tile_pool(name="xt", bufs=1)
        qt_pool = tc.alloc_tile_pool(name="qt", bufs=1)
        kt_pool = tc.alloc_tile_pool(name="kt", bufs=1)
        v_pool = tc.alloc_tile_pool(name="v", bufs=1)
        ot_pool = tc.alloc_tile_pool(name="ot", bufs=1)
        wv_pool = tc.alloc_tile_pool(name="wv", bufs=1)
        w4_pool = tc.alloc_tile_pool(name="w4", bufs=4)
        xn_pool = tc.alloc_tile_pool(name="xn", bufs=6)
        et_pool = tc.alloc_tile_pool(name="et", bufs=1)
        v2_pool = tc.alloc_tile_pool(name="v2", bufs=1)
        c_pool = tc.alloc_tile_pool(name="cp", bufs=1)
        us_pool = tc.alloc_tile_pool(name="us", bufs=2)
        sm_pool = tc.alloc_tile_pool(name="sm", bufs=2)
        y_pool = tc.alloc_tile_pool(name="yp", bufs=2)
        ps_pool = tc.alloc_tile_pool(name="ps", bufs=2, space="PSUM")

        # ---------------- constants ----------------
        ident = const_pool.tile([P, P], FP32, tag="ident")
        make_identity(nc, ident[:])
        # bo row + broadcast (added during the y drain).  Weight/bias DMAs
        # ride the scalar-engine queue so the x stream never waits on them.
        bo_row = const_pool.tile([1, D], FP32, tag="bo_row")
        nc.scalar.dma_start(out=bo_row[:], in_=bo_d.ap()[None, :])
        bo_bc = const_pool.tile([P, D], FP32, tag="bo_bc")
        nc.gpsimd.partition_broadcast(bo_bc[:], bo_row[:])

        # ---------------- persistent intermediates ----------------
        xt = [xt_pool.tile([P, N], FP32R, tag=f"xt{c}", name=f"xt{c}") for c in range(DC)]
        qt = [qt_pool.tile([P, N], FP32R, tag=f"qt{m}", name=f"qt{m}") for m in range(IC)]
        kt = [kt_pool.tile([P, N], FP32R, tag=f"kt{m}", name=f"kt{m}") for m in range(IC)]
        vts = [v_pool.tile([P, INNER], FP32, tag=f"v{j}", name=f"v{j}") for j in range(NB)]
        ot = [ot_pool.tile([P, N], FP32R, tag=f"ot{m}", name=f"ot{m}") for m in range(IC)]

        def load_qk_quarter(key, wd, mb, eng=None):
            w4 = w4_pool.tile([P, DC, P], FP32R, tag="w4", name=f"w4{key}{mb}")
            (eng or nc.scalar).dma_start(
                out=w4[:],
                in_=wd.ap()[:, mb * P:(mb + 1) * P]
                .rearrange("(c p) n -> p c n", p=P).bitcast(FP32R),
            )
            return w4

        # ---------------- preamble ----------------
        w4q = {}

        def emit_x_block(ib, n_warm):
            halves = []
            for hh in range(2):
                xh = xn_pool.tile([P, 512], FP32, tag="xn", name=f"xn{ib}_{hh}")
                nc.sync.dma_start(
                    out=xh[:],
                    in_=x_d.ap()[ib * P:(ib + 1) * P, hh * 512:(hh + 1) * 512],
                )
                halves.append(xh)
            # paced warm-up matmuls on landed data (results unused): keep the
            # PE HAM activity monitor busy through the DMA phase.
            p_w = ps_pool.tile([P, 512], FP32, tag="u", name=f"wu{ib}")
            for w in range(n_warm):
                nc.tensor.matmul(
                    p_w[:, (w % 4) * P:(w % 4 + 1) * P], ident[:],
                    halves[w % 2][:, 0:P], start=True, stop=True,
                )
            p_t = ps_pool.tile([P, N], FP32, tag="big", name=f"ptp{ib}", bufs=3)
            for c in range(DC):
                nc.tensor.transpose(
                    p_t[:, c * P:(c + 1) * P],
                    halves[c // 4][:, (c % 4) * P:(c % 4 + 1) * P],
                    ident[:],
                )
            for c in range(DC):
                if c % 2 == 0:
                    nc.scalar.copy(xt[c][:, ib * P:(ib + 1) * P], p_t[:, c * P:(c + 1) * P])
                else:
                    nc.vector.tensor_copy(xt[c][:, ib * P:(ib + 1) * P], p_t[:, c * P:(c + 1) * P])

        def emit_qk_proj_half(key, dst, mb, ih):
            w4 = w4q[(key, mb)]
            p_t = ps_pool.tile([P, N], FP32, tag="big", name=f"pp{key}{mb}_{ih}", bufs=3)
            for c in range(DC):
                nc.tensor.matmul(
                    p_t[:, 0:512],
                    w4[:, c, :],
                    xt[c][:, ih * 512:(ih + 1) * 512],
                    start=(c == 0), stop=(c == DC - 1),
                )
            nc.vector.tensor_copy(dst[mb][:, ih * 512:(ih + 1) * 512], p_t[:, 0:512])

        def emit_v_proj(jb):
            p_t = ps_pool.tile([P, N], FP32, tag="big", name=f"pv{jb}", bufs=3)
            for c in range(DC):
                nc.tensor.matmul(
                    p_t[:, 0:512],
                    xt[c][:, jb * P:(jb + 1) * P],
                    wv_t[:, c, :],
                    start=(c == 0), stop=(c == DC - 1),
                )
            nc.vector.tensor_copy(vts[jb][:], p_t[:, 0:512])

        w4q[("q", 0)] = load_qk_quarter("q", wq_d, 0)
        w4q[("k", 0)] = load_qk_quarter("k", wk_d, 0)
        wv_t = wv_pool.tile([P, DC, INNER], FP32R, tag="wv")
        nc.scalar.dma_start(
            out=wv_t[:],
            in_=wv_d.ap().rearrange("(c p) n -> p c n", p=P).bitcast(FP32R),
        )
        for ib in range(4):
            emit_x_block(ib, n_warm=4)
        emit_qk_proj_half("q", qt, 0, 0)
        emit_qk_proj_half("k", kt, 0, 0)
        emit_v_proj(0)
        for ib in range(4, 8):
            emit_x_block(ib, n_warm=2)
        emit_v_proj(1)
        emit_qk_proj_half("q", qt, 0, 1)
        emit_qk_proj_half("k", kt, 0, 1)
        w4q[("q", 1)] = load_qk_quarter("q", wq_d, 1)
        w4q[("k", 1)] = load_qk_quarter("k", wk_d, 1)

        # ---------------- head-pair loop ----------------
        finish_args = {}

        def emit_finish(h):
            # 1/R via a PE-transpose reshape: R [1,1024] -> [128,8] so the
            # DVE iterative divide runs on 128 lanes (~0.2us, not ~8us on 1).
            mb, off = h // 2, (h % 2) * DH
            us = finish_args.pop(h)
            pu1 = ps_pool.tile([P, 512], FP32, tag="u", name=f"f1_{h}")
            for ib in range(NB):
                nc.tensor.matmul(
                    pu1[:, ib:ib + 1], us[DH:DH + 1, ib * P:(ib + 1) * P],
                    ident[DH:DH + 1, DH:DH + 1], start=True, stop=True,
                )
            rs = sm_pool.tile([P, 8], FP32, tag="rs", name=f"rs{h}")
            nc.vector.reciprocal(rs[:], pu1[:, 0:8])
            rr = sm_pool.tile([1, N], FP32, tag="rr", name=f"rr{h}", bufs=1)
            for g in range(2):
                pu2 = ps_pool.tile([P, 512], FP32, tag="u", name=f"f2_{h}_{g}")
                for q in range(4):
                    nc.tensor.transpose(
                        pu2[0:1, q * P:(q + 1) * P], rs[:, (g * 4 + q):(g * 4 + q + 1)],
                        ident[:],
                    )
                nc.vector.tensor_copy(rr[:, g * 512:(g + 1) * 512], pu2[0:1, 0:512])
            bc = sm_pool.tile([DH, N], FP32, tag="bc", name=f"bc{h}")
            nc.gpsimd.partition_broadcast(bc[:], rr[:])
            nc.vector.tensor_mul(ot[mb][off:off + DH, :], us[0:DH, :], bc[:])

        pair_state = {}

        def emit_scores(p, jb):
            mb = p
            psA = ps_pool.tile([P, N], FP32, tag="big", name=f"sA{p}_{jb}", bufs=3)
            psB = ps_pool.tile([P, N], FP32, tag="big", name=f"sB{p}_{jb}", bufs=3)
            for ih in range(2):
                nc.tensor.matmul(
                    psA[:, ih * 512:(ih + 1) * 512],
                    kt[mb][0:DH, jb * P:(jb + 1) * P],
                    qt[mb][0:DH, ih * 512:(ih + 1) * 512],
                    start=True, stop=True,
                )
                nc.tensor.matmul(
                    psB[:, ih * 512:(ih + 1) * 512],
                    kt[mb][DH:P, jb * P:(jb + 1) * P],
                    qt[mb][DH:P, ih * 512:(ih + 1) * 512],
                    start=True, stop=True,
                )
            pair_state[(p, jb)] = (psA, psB)

        def emit_chains(p, par):
            """Both i-half attn@V chains for head (2p + par) + GPSIMD drain."""
            st = pair_state[("tiles", p)]
            v2t = st["v2A"] if par == 0 else st["v2B"]
            ett = st["etA"] if par == 0 else st["etB"]
            h = 2 * p + par
            us = us_pool.tile([DH + 1, N], FP32, tag="us", name=f"us{h}")
            for ih in range(2):
                pu = ps_pool.tile([P, 512], FP32, tag="u", name=f"pu{h}_{ih}")
                for jb in range(NB):
                    nc.tensor.matmul(
                        pu[0:DH + 1, :],
                        v2t[:, jb, 0:DH + 1],
                        ett[jb][:, ih * 512:(ih + 1) * 512],
                        start=(jb == 0), stop=(jb == NB - 1),
                    )
                nc.vector.tensor_copy(us[:, ih * 512:(ih + 1) * 512], pu[0:DH + 1, :])
            finish_args[h] = us

        for p in range(4):
            hA, hB = 2 * p, 2 * p + 1
            cA = c_pool.tile([P, NB], FP32, tag="cA", name=f"cA{p}")
            cB = c_pool.tile([P, NB], FP32, tag="cB", name=f"cB{p}")
            v2A = v2_pool.tile([P, NB, DH + 2], BF16, tag="v2A", name=f"v2A{p}")
            v2B = v2_pool.tile([P, NB, DH + 2], BF16, tag="v2B", name=f"v2B{p}")
            etA, etB = {}, {}
            pair_state[("tiles", p)] = {"v2A": v2A, "v2B": v2B, "etA": etA, "etB": etB}

            emit_scores(p, 0)
            emit_scores(p, 1)
            if p > 0:
                emit_chains(p - 1, 1)  # previous pair's B head, under our exps

            for jb in range(NB):
                psA, psB = pair_state.pop((p, jb))
                etA[jb] = et_pool.tile([P, N], BF16, tag=f"etA{jb}", name=f"etA{p}_{jb}")
                nc.scalar.activation(
                    etA[jb][:], psA[:], mybir.ActivationFunctionType.Exp,
                    scale=SCALE, accum_out=cA[:, jb:jb + 1],
                )
                etB[jb] = et_pool.tile([P, N], BF16, tag=f"etB{jb}", name=f"etB{p}_{jb}")
                nc.scalar.activation(
                    etB[jb][:], psB[:], mybir.ActivationFunctionType.Exp,
                    scale=SCALE, accum_out=cB[:, jb:jb + 1],
                )
                nc.gpsimd.normalize_recip(
                    v2A[:, jb, 0:DH], vts[jb][:, hA * DH:(hA + 1) * DH], cA[:, jb:jb + 1]
                )
                nc.gpsimd.tensor_copy(v2A[:, jb, DH:DH + 1], cA[:, jb:jb + 1])
                nc.gpsimd.normalize_recip(
                    v2B[:, jb, 0:DH], vts[jb][:, hB * DH:(hB + 1) * DH], cB[:, jb:jb + 1]
                )
                nc.gpsimd.tensor_copy(v2B[:, jb, DH:DH + 1], cB[:, jb:jb + 1])

                if jb < 6:
                    emit_scores(p, jb + 2)

                # PE filler
                if p == 0:
                    if jb < 6:
                        emit_v_proj(jb + 2)
                    if jb == 3:
                        emit_qk_proj_half("q", qt, 1, 0)
                    if jb == 4:
                        emit_qk_proj_half("k", kt, 1, 0)
                    if jb == 5:
                        emit_qk_proj_half("q", qt, 1, 1)
                elif p < 3:
                    if jb == 0:
                        emit_qk_proj_half("k", kt, p, 1)  # prev pair's spill
                    if jb == 1:
                        emit_qk_proj_half("q", qt, p + 1, 0)
                    if jb == 3:
                        emit_qk_proj_half("k", kt, p + 1, 0)
                    if jb == 5:
                        emit_qk_proj_half("q", qt, p + 1, 1)
                else:
                    if jb == 0:
                        emit_qk_proj_half("k", kt, 3, 1)

                if p > 0 and jb == 1:
                    emit_finish(2 * p - 2)
                if p > 0 and jb == 5:
                    emit_finish(2 * p - 1)

            emit_chains(p, 0)  # A head's chains, under B's exp stream

            if p == 0:
                w4q[("q", 2)] = load_qk_quarter("q", wq_d, 2, eng=nc.sync)
                w4q[("k", 2)] = load_qk_quarter("k", wk_d, 2, eng=nc.sync)
            if p == 1:
                w4q[("q", 3)] = load_qk_quarter("q", wq_d, 3, eng=nc.sync)
                w4q[("k", 3)] = load_qk_quarter("k", wk_d, 3, eng=nc.sync)

        # Wo quarters into freed w4 slots (sync queue, idle post-preamble)
        wo4 = []
        for mbi in range(IC):
            w4 = w4_pool.tile([P, D], FP32R, tag="w4", name=f"w4o{mbi}")
            nc.sync.dma_start(
                out=w4[:], in_=wo_d.ap()[mbi * P:(mbi + 1) * P, :].bitcast(FP32R)
            )
            wo4.append(w4)

        # ---------------- tail ----------------
        emit_finish(H - 2)        # head 6; DVE recip runs under the B chains
        emit_chains(3, 1)         # head 7 chains
        # output projection: first 3 blocks start on mb0-2 before head 7's
        # normalization lands; mb3 links + the rest follow.
        pys = {}
        for ib in range(3):
            p_y = ps_pool.tile([P, N], FP32, tag="big", name=f"py{ib}", bufs=3)
            for db in range(2):
                for mbi in range(3):
                    nc.tensor.matmul(
                        p_y[:, db * 512:(db + 1) * 512],
                        ot[mbi][:, ib * P:(ib + 1) * P],
                        wo4[mbi][:, db * 512:(db + 1) * 512],
                        start=(mbi == 0), stop=False,
                    )
            pys[ib] = p_y
        emit_finish(H - 1)        # head 7

        def drain_y(ib, p_y):
            for db in range(2):
                y_t = y_pool.tile([P, 512], FP32, tag="y", name=f"y{ib}_{db}")
                nc.vector.tensor_add(
                    y_t[:], p_y[:, db * 512:(db + 1) * 512],
                    bo_bc[:, db * 512:(db + 1) * 512],
                )
                nc.sync.dma_start(
                    out=y_d.ap()[ib * P:(ib + 1) * P, db * 512:(db + 1) * 512],
                    in_=y_t[:],
                )

        for ib in range(3):
            p_y = pys.pop(ib)
            for db in range(2):
                nc.tensor.matmul(
                    p_y[:, db * 512:(db + 1) * 512],
                    ot[3][:, ib * P:(ib + 1) * P],
                    wo4[3][:, db * 512:(db + 1) * 512],
                    start=False, stop=True,
                )
            drain_y(ib, p_y)
        for ib in range(3, NB):
            p_y = ps_pool.tile([P, N], FP32, tag="big", name=f"py{ib}", bufs=3)
            for db in range(2):
                for mbi in range(IC):
                    nc.tensor.matmul(
                        p_y[:, db * 512:(db + 1) * 512],
                        ot[mbi][:, ib * P:(ib + 1) * P],
                        wo4[mbi][:, db * 512:(db + 1) * 512],
                        start=(mbi == 0), stop=(mbi == IC - 1),
                    )
            drain_y(ib, p_y)

        for pool in (ps_pool, y_pool, sm_pool, us_pool, c_pool, v2_pool, et_pool,
                     xn_pool, w4_pool, wv_pool, ot_pool, v_pool, kt_pool, qt_pool,
                     xt_pool, const_pool):
            pool.release()

    nc.finalize()
    return nc


def _get_nc():
    global _NC_CACHE
    if _NC_CACHE is None:
        _NC_CACHE = _build_nc()
    return _NC_CACHE


def kernel(x, Wq, Wk, Wv, Wo, bo, _trace=False, **trace_kwargs):
    x = np.ascontiguousarray(np.asarray(x, dtype=np.float32))
    Wq = np.ascontiguousarray(np.asarray(Wq, dtype=np.float32))
    Wk = np.ascontiguousarray(np.asarray(Wk, dtype=np.float32))
    Wv = np.ascontiguousarray(np.asarray(Wv, dtype=np.float32))
    Wo = np.ascontiguousarray(np.asarray(Wo, dtype=np.float32))
    bo = np.ascontiguousarray(np.asarray(bo, dtype=np.float32))

    nc = _get_nc()
    in_maps = [
        {"x": x[c], "Wq": Wq, "Wk": Wk, "Wv": Wv, "Wo": Wo, "bo": bo}
        for c in range(NCORES)
    ]
    res = run_bass_kernel_spmd(
        nc, in_maps, core_ids=list(range(NCORES)), trace=_trace, **trace_kwargs
    )
    out = np.stack([res.results[c]["y"] for c in range(NCORES)], axis=0)
    if _trace:
        return out.astype(np.float32), res
    return out.astype(np.float32)


if __name__ == "__main__":
    rng = np.random.default_rng(0)
    xs = rng.standard_normal((B, N, D), dtype=np.float32)
    wq = rng.standard_normal((D, INNER), dtype=np.float32) * D ** -0.5
    wk = rng.standard_normal((D, INNER), dtype=np.float32) * D ** -0.5
    wv = rng.standard_normal((D, INNER), dtype=np.float32) * D ** -0.5
    wo = rng.standard_normal((INNER, D), dtype=np.float32) * INNER ** -0.5
    bz = np.zeros((D,), dtype=np.float32)
    y = kernel(xs, wq, wk, wv, wo, bz)
    print("ran ok", y.shape, float(np.abs(y).mean()))
